# revision 14
# baseline (speedup 1.0000x reference)
"""Trainium2 Bass kernel for nn_Attention_23759759081800.

Fused attention block: qkv proj + QK-LayerNorm + LPE (per-channel affine on v)
+ softmax attention + output proj, for x (2, 2048, 1024), 16 heads, d=64.

Sharding over 8 NeuronCores: data-parallel over batch (2 groups of 4 cores)
x head-parallel (4 heads per core). LayerNorm stats (over the full 1024
channels) need a tiny AllReduce within each 4-core group; the output proj
contracts over all channels, redistributed with one 8-rank AllToAll that also
splits proj work into 256-token blocks per core.

All matmuls run as float32r (e8m11, 1 cycle/row) with fp32 accumulation.
Softmax skips the max-subtraction pass (logits are LayerNorm-bounded, exp in
fp32 cannot overflow) and folds the row-sum into the AV matmul via an
appended ones-column on V; normalization by 1/sumexp happens on the (N,64)
output instead of the (N,N) matrix.
"""
import sys

if "/opt/trn_rl_repo" not in sys.path:
    sys.path.insert(0, "/opt/trn_rl_repo")

import numpy as np
import concourse.bass as bass
import concourse.mybir as mybir
import concourse.tile as tile
from concourse import bacc
from concourse.bass import ts
from concourse.bass_utils import run_bass_kernel_spmd

F32 = mybir.dt.float32
F32R = mybir.dt.float32r
AF = mybir.ActivationFunctionType
ALU = mybir.AluOpType

B, C, H, D = 2, 1024, 16, 64
N_TOK_FULL = 2048
EPS = 1e-5
P = 128
LOCF = 256  # channels per core
GROUPS = [[0, 1, 2, 3], [4, 5, 6, 7]]


def round_fp32r(x: np.ndarray) -> np.ndarray:
    """Round fp32 to fp32r (e8m11: top 20 bits), round-to-nearest-even-ish."""
    v = np.ascontiguousarray(x, dtype=np.float32).view(np.uint32)
    r = v + 0x7FF + ((v >> 12) & 1)
    r &= np.uint32(0xFFFFF000)
    return r.view(np.float32)


def build_nc(n_tok: int = N_TOK_FULL, collectives: bool = True):
    """Build the SPMD program (identical on all 8 cores)."""
    NCH = n_tok // 512   # 512-token chunks
    KT = n_tok // 128    # 128-token key tiles
    TOKT = n_tok // 128
    SHARD = n_tok // 8   # tokens per a2a shard
    MT = SHARD // 128    # proj m-tiles per batch-part

    nc = bacc.Bacc("TRN2", target_bir_lowering=False, debug=False, num_devices=8)

    xt_e = nc.dram_tensor("xt", [C, n_tok], F32R, kind="ExternalInput")
    wt_e = nc.dram_tensor("wt", [C, 3 * LOCF], F32R, kind="ExternalInput")
    wpt_e = nc.dram_tensor("wpt", [C, C], F32R, kind="ExternalInput")
    coefs_e = nc.dram_tensor("coefs", [P, 12], F32, kind="ExternalInput")
    biasb_e = nc.dram_tensor("biasb", [P, C], F32, kind="ExternalInput")
    y_e = nc.dram_tensor("y", [2, SHARD, C], F32, kind="ExternalOutput")

    xt_ap = xt_e.ap().rearrange("(o p) t -> p o t", p=P)     # [128, 8, n_tok]
    wt_ap = wt_e.ap().rearrange("(o p) f -> p o f", p=P)     # [128, 8, 768]
    wpt_ap = wpt_e.ap().rearrange("(o p) c -> p o c", p=P)   # [128, 8, 1024]

    with tile.TileContext(nc) as tc:
        with (
            tc.tile_pool(name="const", bufs=1) as cpool,
            tc.tile_pool(name="qk", bufs=1) as qkpool,
            tc.tile_pool(name="va", bufs=1) as vapool,
            tc.tile_pool(name="stats", bufs=1) as stpool,
            tc.tile_pool(name="tmps", bufs=3) as tmps,
            tc.tile_pool(name="et", bufs=3) as etpool,
            tc.tile_pool(name="psA", bufs=2, space="PSUM") as psA,
            tc.tile_pool(name="psMM", bufs=2, space="PSUM") as psMM,
            tc.tile_pool(name="psST", bufs=2, space="PSUM") as psST,
            tc.tile_pool(name="dram", bufs=1, space="DRAM") as dram,
        ):
            # ---- constants ----
            coefs = cpool.tile([P, 2, 6], F32)
            nc.sync.dma_start(coefs[:], coefs_e.ap().rearrange("p (f s) -> p f s", f=2))
            biasb = cpool.tile([P, C], F32)
            nc.sync.dma_start(biasb[:], biasb_e[:])
            ones128 = cpool.tile([P, 1], F32R)
            nc.any.memset(ones128[:].bitcast(F32), 1.0)
            ones1 = cpool.tile([1, P], F32R)
            nc.any.memset(ones1[:].bitcast(F32), 1.0)
            epst = cpool.tile([1, 1], F32)
            nc.any.memset(epst[:], EPS)
            invc1 = cpool.tile([1, P], F32R)
            nc.any.memset(invc1[:].bitcast(F32), 1.0 / C)

            # ---- persistent activations ----
            qf = qkpool.tile([P, 2, n_tok], F32R)   # q feature-major [f, fo, tok]
            kf = qkpool.tile([P, 2, n_tok], F32R)
            vaug = vapool.tile([P, TOKT, 4, D + 1], F32R)  # [tok, to, head, d|1]
            resid = vapool.tile([P, 2, n_tok], F32R)       # v_lpe + attn out
            nc.any.memset(vaug[:, :, :, D : D + 1].bitcast(F32), 1.0)



            # ============ P1: q,k projection + local stats ============
            with (
                tc.tile_pool(name="wtp", bufs=1) as wtpool,
                tc.tile_pool(name="xtp", bufs=2) as xtpool,
            ):
                wt = wtpool.tile([P, 8, 3 * LOCF], F32R)
                nc.sync.dma_start(wt[:], wt_ap[:])
                st_in = dram.tile([4, n_tok], F32)
                st_out = dram.tile([4, n_tok], F32)
                for nch in range(NCH):
                    xch = xtpool.tile([P, 8, 512], F32R, tag="xch")
                    nc.sync.dma_start(xch[:], xt_ap[:, :, ts(nch, 512)])
                    for fo in range(4):  # 2 q m-tiles then 2 k m-tiles
                        ps = psMM.tile([P, 512], F32, tag="mm")
                        for co in range(8):
                            nc.tensor.matmul(
                                ps[:],
                                wt[:, co, ts(fo, P)],
                                xch[:, co, :],
                                start=(co == 0),
                                stop=(co == 7),
                            )
                        dest = qf if fo < 2 else kf
                        nc.scalar.copy(dest[:, fo % 2, ts(nch, 512)], ps[:])
                    # local stats for this chunk
                    for si in range(4):
                        qk3, issq = si // 2, si % 2
                        feat3 = qf if qk3 == 0 else kf
                        pstat = psST.tile([1, 512], F32, tag="stat")
                        for fo in range(2):
                            if issq:
                                sq3 = tmps.tile([P, 512], F32R, tag="sq")
                                nc.vector.tensor_tensor(
                                    sq3[:],
                                    feat3[:, fo, ts(nch, 512)],
                                    feat3[:, fo, ts(nch, 512)],
                                    ALU.mult,
                                )
                                msrc = sq3[:]
                            else:
                                msrc = feat3[:, fo, ts(nch, 512)]
                            nc.tensor.matmul(
                                pstat[:],
                                ones128[:],
                                msrc,
                                start=(fo == 0),
                                stop=(fo == 1),
                            )
                        stg = tmps.tile([1, 512], F32, tag="stg")
                        nc.vector.tensor_copy(stg[:], pstat[:])
                        nc.sync.dma_start(st_in[2 * qk3 + issq, ts(nch, 512)], stg[:])

                # ============ P2: stats AllReduce over the batch group ======
                if collectives:
                    nc.gpsimd.collective_compute(
                        "AllReduce",
                        ALU.add,
                        replica_groups=GROUPS,
                        ins=[st_in.opt()],
                        outs=[st_out.opt()],
                    )
                else:
                    nc.gpsimd.dma_start(st_out[:], st_in[:])

                # ============ P3: v projection (both layouts) ============
                for nch in range(NCH):
                    xch = xtpool.tile([P, 8, 512], F32R, tag="xch")
                    nc.sync.dma_start(xch[:], xt_ap[:, :, ts(nch, 512)])
                    # token-major v -> vaug (AV stationary)
                    for tj in range(4):
                        to = nch * 4 + tj
                        psv_full = psMM.tile([P, 512], F32, tag="mm")
                        psv = psv_full[:, 0:256]
                        for co in range(8):
                            nc.tensor.matmul(
                                psv[:],
                                xch[:, co, ts(tj, P)],
                                wt[:, co, 512:768],
                                start=(co == 0),
                                stop=(co == 7),
                            )
                        nc.vector.tensor_copy(
                            vaug[:, to, :, 0:D],
                            psv[:].rearrange("p (h d) -> p h d", h=4),
                        )
                    # feature-major v -> resid with LPE affine
                    for fo in range(2):
                        ps = psMM.tile([P, 512], F32, tag="mm")
                        for co in range(8):
                            nc.tensor.matmul(
                                ps[:],
                                wt[:, co, ts(4 + fo, P)],
                                xch[:, co, :],
                                start=(co == 0),
                                stop=(co == 7),
                            )
                        nc.scalar.activation(
                            resid[:, fo, ts(nch, 512)],
                            ps[:],
                            AF.Identity,
                            scale=coefs[:, fo, 4:5],
                            bias=coefs[:, fo, 5:6],
                        )

            # ============ P4: center q,k by mu; compute ln(var) rows ========
            # (two ACT emission passes -- all Ln's, then all Exp's -- so the
            # activation table set switches once, not per chunk)
            inv_c = 1.0 / C
            lnvars = {}
            for nch in range(NCH):
                pbm = psA.tile([P, 2, 512], F32, tag="sc")
                for qk2 in range(2):
                    g0 = tmps.tile([1, 512], F32R, tag="drow")
                    g1 = tmps.tile([1, 512], F32, tag="drow")
                    nc.sync.dma_start(g0[:].bitcast(F32), st_out[2 * qk2, ts(nch, 512)])
                    nc.sync.dma_start(g1[:], st_out[2 * qk2 + 1, ts(nch, 512)])
                    # broadcast mu = sum/C via (1/C)-valued lhsT outer product
                    nc.tensor.matmul(
                        pbm[:, qk2, :], invc1[:], g0[:], start=True, stop=True
                    )
                    lv = stpool.tile([1, 512], F32, name=f"lv_{nch}_{qk2}")
                    t1 = tmps.tile([1, 512], F32, tag="drow")
                    t2 = tmps.tile([1, 512], F32, tag="drow")
                    nc.vector.tensor_scalar_mul(t1[:], g1[:], inv_c)
                    nc.vector.tensor_scalar_mul(t2[:], g0[:], inv_c)
                    nc.vector.tensor_tensor(t2[:], t2[:], t2[:], ALU.mult)
                    nc.vector.tensor_tensor(t1[:], t1[:], t2[:], ALU.subtract)
                    nc.scalar.activation(lv[:], t1[:], AF.Ln, bias=epst[:])
                    lnvars[(nch, qk2)] = lv
                for qk, feat in ((0, qf), (1, kf)):
                    for fo in range(2):
                        nc.vector.tensor_tensor(
                            feat[:, fo, ts(nch, 512)],
                            feat[:, fo, ts(nch, 512)],
                            pbm[:, qk, :],
                            ALU.subtract,
                        )

            # ============ P5: rstd (Exp), broadcast, scale q,k ==============
            for nch in range(NCH):
                pbr = psA.tile([P, 2, 512], F32, tag="sc")
                for qk2 in range(2):
                    rs = tmps.tile([1, 512], F32R, tag="drow")
                    nc.scalar.activation(rs[:], lnvars[(nch, qk2)][:], AF.Exp, scale=-0.5)
                    nc.tensor.matmul(pbr[:, qk2, :], ones1[:], rs[:], start=True, stop=True)
                for qk, feat in ((0, qf), (1, kf)):
                    for fo in range(2):
                        t2 = tmps.tile([P, 512], F32, tag="nrm")
                        nc.vector.tensor_tensor(
                            t2[:], feat[:, fo, ts(nch, 512)], pbr[:, qk, :], ALU.mult
                        )
                        nc.vector.tensor_scalar(
                            feat[:, fo, ts(nch, 512)],
                            t2[:],
                            coefs[:, fo, 2 * qk : 2 * qk + 1],
                            coefs[:, fo, 2 * qk + 1 : 2 * qk + 2],
                            ALU.mult,
                            ALU.add,
                        )

            # ============ P6: attention, head-pairs fused ============
            # Heads 2*fo and 2*fo+1 live on disjoint 64-row groups (base
            # partition 0 / 64), so their K=64 score matmuls can overlap on
            # the PE via per-subarray row tiling; exp batches both heads.
            for fo in range(2):
                for qc in range(NCH):
                    psav0 = psMM.tile([P, 512], F32, tag="mm")
                    psav1 = psMM.tile([P, 512], F32, tag="mm")
                    psavs = (psav0, psav1)
                    for kt in range(KT):
                        psc = psA.tile([P, 2, 512], F32, tag="sc")
                        for b2 in range(2):
                            nc.tensor.matmul(
                                psc[:, b2, :],
                                kf[64 * b2 : 64 * b2 + 64, fo, ts(kt, P)],
                                qf[64 * b2 : 64 * b2 + 64, fo, ts(qc, 512)],
                                start=True,
                                stop=True,
                            )
                        et = etpool.tile([P, 2, 512], F32R, tag="et")
                        nc.scalar.activation(et[:], psc[:], AF.Exp)
                        for b2 in range(2):
                            h2 = 2 * fo + b2
                            nc.tensor.matmul(
                                psavs[b2][0 : D + 1, :],
                                vaug[:, kt, h2, :],
                                et[:, b2, :],
                                start=(kt == 0),
                                stop=(kt == KT - 1),
                            )
                    for b2 in range(2):
                        psav = psavs[b2]
                        pbase = 64 * b2
                        rc = tmps.tile([1, 512], F32R, tag="rc")
                        with nc.allow_low_precision(reason="softmax 1/sumexp as fp32r"):
                            nc.vector.reciprocal(rc[:], psav[D : D + 1, :])
                        prc = psST.tile([P, 512], F32, tag="stat")
                        nc.tensor.matmul(prc[:], ones1[:], rc[:], start=True, stop=True)
                        t_oc = tmps.tile([P, 512], F32, tag="onrm")
                        nc.vector.tensor_copy(t_oc[pbase : pbase + D, :], psav[0:D, :])
                        t_o = tmps.tile([P, 512], F32, tag="onrm")
                        nc.vector.tensor_tensor(
                            t_o[pbase : pbase + D, :],
                            t_oc[pbase : pbase + D, :],
                            prc[pbase : pbase + D, :],
                            ALU.mult,
                        )
                        nc.vector.tensor_tensor(
                            resid[pbase : pbase + D, fo, ts(qc, 512)],
                            resid[pbase : pbase + D, fo, ts(qc, 512)],
                            t_o[pbase : pbase + D, :],
                            ALU.add,
                        )

            # ============ P7: AllToAll redistribution ============
            a2a_in = dram.tile([8, LOCF, SHARD], F32R)
            a2a_out = dram.tile([8, LOCF, SHARD], F32R)
            for j in range(8):
                nc.sync.dma_start(
                    a2a_in[j].rearrange("(f p) t -> p f t", p=P),
                    resid[:, :, ts(j, SHARD)],
                )
            if collectives:
                nc.gpsimd.collective_compute(
                    "AllToAll",
                    ALU.bypass,
                    replica_groups=[list(range(8))],
                    ins=[a2a_in.opt()],
                    outs=[a2a_out.opt()],
                )
            else:
                nc.gpsimd.dma_start(a2a_out[:], a2a_in[:])

            # ============ P8: output projection ============
            with (
                tc.tile_pool(name="wp", bufs=1) as wppool,
                tc.tile_pool(name="pj", bufs=2) as pjpool,
                tc.tile_pool(name="yp", bufs=3) as ypool,
            ):
                wpt = wppool.tile([P, 8, C], F32R)
                nc.sync.dma_start(wpt[:], wpt_ap[:])
                for bp in range(2):
                    pjb = pjpool.tile([P, 8, SHARD], F32R, tag="pjb")
                    for j in range(4):
                        nc.sync.dma_start(
                            pjb[:, 2 * j : 2 * j + 2, :],
                            a2a_out[4 * bp + j].rearrange("(ci p) t -> p ci t", p=P),
                        )
                    for mt in range(MT):
                        for nch2 in range(2):
                            psy = psMM.tile([P, 512], F32, tag="mm")
                            for jc in range(8):
                                nc.tensor.matmul(
                                    psy[:],
                                    pjb[:, jc, ts(mt, P)],
                                    wpt[:, jc, ts(nch2, 512)],
                                    start=(jc == 0),
                                    stop=(jc == 7),
                                )
                            yt = ypool.tile([P, 512], F32, tag="yt")
                            nc.vector.tensor_tensor(
                                yt[:], psy[:], biasb[:, ts(nch2, 512)], ALU.add
                            )
                            nc.sync.dma_start(y_e[bp, ts(mt, P), ts(nch2, 512)], yt[:])

    nc.compile()
    return nc


def prep_in_maps(
    x, w_qkv, q_gamma, q_beta, k_gamma, k_beta, lpe_w, lpe_b, w_proj, b_proj,
    n_tok: int = N_TOK_FULL,
):
    """Shard the full inputs into the 8 per-core input maps."""
    x = np.asarray(x, np.float32)
    w_qkv = np.asarray(w_qkv, np.float32)
    w_proj = np.asarray(w_proj, np.float32)
    vecs = [np.asarray(v, np.float32) for v in
            (q_gamma, q_beta, k_gamma, k_beta, lpe_w, lpe_b, b_proj)]
    q_gamma, q_beta, k_gamma, k_beta, lpe_w, lpe_b, b_proj = vecs

    scale = float(D) ** -0.5
    wq, wk, wv = w_qkv[0:C], w_qkv[C : 2 * C], w_qkv[2 * C : 3 * C]
    wpt = round_fp32r(np.ascontiguousarray(w_proj.T))
    biasb = np.ascontiguousarray(np.broadcast_to(b_proj, (P, C)))

    in_maps = []
    for c in range(8):
        b_, hg = c // 4, c % 4
        sl = slice(LOCF * hg, LOCF * hg + LOCF)
        xt = round_fp32r(np.ascontiguousarray(x[b_, :n_tok].T))
        wt = round_fp32r(
            np.concatenate([wq[sl].T, wk[sl].T, wv[sl].T], axis=1)
        )
        def two(v):
            return v[sl].reshape(2, P).T  # [pi, fo]
        coefs = np.stack(
            [
                two(q_gamma) * scale,
                two(q_beta) * scale,
                two(k_gamma),
                two(k_beta),
                two(lpe_w),
                two(lpe_b),
            ],
            axis=-1,
        )  # [128, 2, 6]
        coefs = np.ascontiguousarray(coefs.reshape(P, 12), np.float32)
        in_maps.append(
            {"xt": xt, "wt": wt, "wpt": wpt, "coefs": coefs, "biasb": biasb}
        )
    return in_maps


def assemble_y(results, n_tok: int = N_TOK_FULL) -> np.ndarray:
    SHARD = n_tok // 8
    y = np.empty((B, n_tok, C), np.float32)
    for c in range(8):
        yc = results[c]["y"]
        y[0, SHARD * c : SHARD * (c + 1)] = yc[0]
        y[1, SHARD * c : SHARD * (c + 1)] = yc[1]
    return y


_NC_CACHE = {}


def kernel(**inputs) -> np.ndarray:
    key = ("full", N_TOK_FULL)
    if key not in _NC_CACHE:
        _NC_CACHE[key] = build_nc(N_TOK_FULL, collectives=True)
    nc = _NC_CACHE[key]
    in_maps = prep_in_maps(**inputs)
    res = run_bass_kernel_spmd(nc, in_maps, core_ids=list(range(8)))
    return assemble_y(res.results)
